# revision 44
# baseline (speedup 1.0000x reference)
"""Encoder-decoder cross-attention (T5-style additive position bias) on 8 TRN2
NeuronCores.

Reference computation (per batch element b):
    Q = x @ Wq.T,  K = enc @ Wk.T,  V = enc @ Wv.T          (split into 8 heads of 64)
    scores = Q_h @ K_h.T + pos_bias[h]                       [NQ, NKV]
    out_h = softmax(scores) @ V_h
    out = concat_h(out_h) @ Wo.T

Sharding: pure data parallel over the batch (B=8 -> one batch element per core).
Each core streams the full 64 MB pos_bias (the dominant DMA term); compute is
sized under the DMA roofline, so the kernel targets the memory roofline.

Device-side layout: everything is computed transposed-by-head so the softmax'd
scores come out with the NKV (contraction) axis on partitions, feeding the
attn @ V matmul directly with zero on-chip transposes:
    QT[d, q] = Wq @ x.T        (host passes xT, WqT)
    KT[d, k] = Wk @ enc.T
    V[k, d]  = enc @ Wv.T      (+ a ones column per head -> softmax denominator)
    scT_h[k, q] = QK matmul accumulated in PSUM, bias added either by an
                  identity-matmul PSUM-accumulate on PE (1/4 of tiles) or a
                  DVE tensor_add into SBUF (3/4) to balance engine load.
                  The per-head Q/K rows are duplicated into both partition
                  halves (SWDGE copies, one head ahead) so the QK matmuls for
                  the two k-tiles of each pair run row-packed (tile_position
                  (0,0)/(64,0)) CONCURRENTLY in the PE array - the K=64
                  contraction would otherwise idle half the array
    accT_h[d(+1), q] = sum_k V'_h[k, d] * exp(scT_h[k, q])   (row 64 = denom)
    out[q, :] = (accT / denom) contracted with WoT

The QT/KT/V projections are EMITTED INTERLEAVED into the head loop (V inside
head 0 with one-pair prefetch, QT/KT for pair p spread across head 2p-1) —
Tile schedules engines in program-priority order, so this keeps the PE stream
and the PSUM slot queue flowing with the bias-stream pipeline instead of
front-running all projections.

QK and the projections run as float32r (~1.6e-4 max rel err measured on HW,
4x faster than float32); the position bias streams as fp16 (2^-11 quantization
of O(1) values, halving the dominant DMA term); exp outputs, V, attnT and Wo
are fp16 (softmax weights and O(0.5) values). The exp is computed with a
constant -8 shift folded into the ACT affine (cancels in the softmax ratio)
so fp16 exp cannot overflow; otherwise no max-subtraction pass is needed
(scores are O(10), far from fp32 exp range). Measured end-to-end vs the fp32
reference: ~6e-4 relative error.
"""

import os
import sys

for _p in ("/opt/trn_rl_repo", "/root/.axon_site/_ro/trn_rl_repo"):
    if _p not in sys.path and os.path.isdir(_p):
        sys.path.append(_p)

import numpy as np

import concourse.bass as bass
import concourse.tile as tile
from concourse import bacc, mybir
from concourse.bass_utils import run_bass_kernel_spmd
from concourse.masks import make_identity

B, NQ, NKV, D, H, HD = 8, 1024, 2048, 512, 8, 64
P = 128
F32 = mybir.dt.float32
F32R = mybir.dt.float32r
F16 = mybir.dt.float16
EXP = mybir.ActivationFunctionType.Exp


def build(nq=NQ, nkv=NKV, d=D, h=H, hd=HD, pe_add_mod=3):
    nDt = d // P          # dout partition tiles (4)
    nKt = nkv // P        # k partition tiles per head (16)
    QF = min(512, nq)     # matmul moving free-dim chunk
    nQc = nq // QF        # q chunks (2)
    KF = min(512, nkv)
    hpt = P // hd         # heads per 128-row tile (2)
    hw = hd + 1           # head width incl. ones column
    npair = nKt // 2

    nc = bacc.Bacc("TRN2")
    xT = nc.dram_tensor("xT", [d, nq], F32R, kind="ExternalInput")
    encT = nc.dram_tensor("encT", [d, nkv], F32R, kind="ExternalInput")
    biasT = nc.dram_tensor("biasT", [h, nkv, nq], F16, kind="ExternalInput")
    wqT = nc.dram_tensor("wqT", [d, d], F32R, kind="ExternalInput")
    wkT = nc.dram_tensor("wkT", [d, d], F32R, kind="ExternalInput")
    wvT = nc.dram_tensor("wvT", [d, d], F32R, kind="ExternalInput")
    woT = nc.dram_tensor("woT", [d, d], F32R, kind="ExternalInput")
    out = nc.dram_tensor("out", [nq, d], F32, kind="ExternalOutput")

    with tile.TileContext(nc) as tc:
        with (
            tc.tile_pool(name="persist", bufs=1) as persist,
            tc.tile_pool(name="qtp", bufs=2) as qtp,
            tc.tile_pool(name="qthp", bufs=2) as qthp,
            tc.tile_pool(name="kthp", bufs=2) as kthp,
            tc.tile_pool(name="ktp", bufs=2) as ktp,
            tc.tile_pool(name="bias", bufs=3) as bias_pool,
            tc.tile_pool(name="expp", bufs=4) as exp_pool,
            tc.tile_pool(name="sadd", bufs=4) as sadd_pool,
            tc.tile_pool(name="small", bufs=1) as small,
            tc.tile_pool(name="outp", bufs=2) as outp,
            tc.tile_pool(name="ps", bufs=3, space="PSUM") as ps,
            tc.tile_pool(name="ps_acc", bufs=1, space="PSUM") as ps_acc,
            tc.tile_pool(name="ph1", bufs=1) as ph1,
        ):
            vx = [persist.tile([P, h * hw], F16, name=f"vx{j}") for j in range(nKt)]
            wo = [persist.tile([P, d], F16, name=f"wo{j}") for j in range(nDt)]
            attnT = [persist.tile([P, nq], F16, name=f"attnT{j}") for j in range(nDt)]
            ident_f32 = persist.tile([P, P], F32, name="ident_f32")
            ident = persist.tile([P, P], F16, name="ident")
            make_identity(nc, ident_f32[:])
            nc.vector.tensor_copy(ident[:], ident_f32[:])

            xt = [ph1.tile([P, nq], F32R, name=f"xt{j}") for j in range(nDt)]
            et = [ph1.tile([P, nkv], F32R, name=f"et{j}") for j in range(nDt)]
            wq = [ph1.tile([P, d], F32R, name=f"wq{j}") for j in range(nDt)]
            wk = [ph1.tile([P, d], F32R, name=f"wk{j}") for j in range(nDt)]
            wv = [ph1.tile([P, d], F32R, name=f"wv{j}") for j in range(nDt)]
            ones_src = ph1.tile([P, h], F32, name="ones_src")
            nc.vector.memset(ones_src[:], 1.0)
            shift = persist.tile([P, 1], F32, name="shift")
            nc.vector.memset(shift[:], -8.0)

            # sync ring: small critical weights first, then the bias stream
            # owns it. scalar ring: xt+wv (needed ~10 us in). gpsimd SWDGE:
            # enc (overlaps the weight loads), wo (phase 3), output stores.
            for j in range(nDt):
                nc.sync.dma_start(wk[j][:], wkT[j * P:(j + 1) * P, :])
            for j in range(nDt):
                nc.sync.dma_start(wq[j][:], wqT[j * P:(j + 1) * P, :])
            for j in range(nDt):
                nc.gpsimd.dma_start(et[j][:], encT[j * P:(j + 1) * P, :])
            for j in range(nDt):
                nc.sync.dma_start(xt[j][:], xT[j * P:(j + 1) * P, :])
            for j in range(nDt):
                nc.scalar.dma_start(wv[j][:], wvT[j * P:(j + 1) * P, :])
            for j in range(nDt):
                nc.gpsimd.dma_start(wo[j][:], woT[j * P:(j + 1) * P, :])

            def emit_qt(dt):
                # QT[dout, q] = Wq @ xT : lhsT = wqT[din, dout], rhs = xT[din, q]
                t = qtp.tile([P, nq], F32R, tag="qt", name="qt")
                pt = ps.tile([P, nq], F32, tag="ps", name="pt")
                for k in range(nDt):
                    for qc in range(nQc):
                        nc.tensor.matmul(
                            pt[:, qc * QF:(qc + 1) * QF],
                            wq[k][:, dt * P:(dt + 1) * P],
                            xt[k][:, qc * QF:(qc + 1) * QF],
                            start=(k == 0),
                            stop=(k == nDt - 1),
                        )
                nc.vector.tensor_copy(t[:], pt[:])
                return t

            def emit_kt_chunk(t, dt, kc2):
                # KT[dout, k] = Wk @ encT, one nq-wide chunk of k
                pt = ps.tile([P, nq], F32, tag="ps", name="pt")
                for k in range(nDt):
                    for sub in range(nq // KF):
                        off = kc2 * nq + sub * KF
                        nc.tensor.matmul(
                            pt[:, sub * KF:(sub + 1) * KF],
                            wk[k][:, dt * P:(dt + 1) * P],
                            et[k][:, off:off + KF],
                            start=(k == 0),
                            stop=(k == nDt - 1),
                        )
                nc.vector.tensor_copy(t[:, kc2 * nq:(kc2 + 1) * nq], pt[:])

            def emit_v_pair(pi):
                # V[n, dout] = enc @ Wv.T for n-tiles 2*pi, 2*pi+1 (one psum tile)
                pt = ps.tile([P, nq], F32, tag="ps", name="pt")
                for half in range(2):
                    nt = 2 * pi + half
                    for k in range(nDt):
                        nc.tensor.matmul(
                            pt[:, half * d:half * d + d],
                            et[k][:, nt * P:(nt + 1) * P],
                            wv[k][:],
                            start=(k == 0),
                            stop=(k == nDt - 1),
                        )
                for half in range(2):
                    nt = 2 * pi + half
                    vg = vx[nt][:].rearrange("p (h e) -> p h e", e=hw)
                    nc.vector.tensor_copy(
                        vg[:, :, hd:hw], ones_src[:].unsqueeze(-1)
                    )
                    nc.vector.tensor_copy(
                        vg[:, :, 0:hd],
                        pt[:, half * d:half * d + d].rearrange(
                            "p (h e) -> p h e", e=hd
                        ),
                    )

            def emit_dup(src_qt, src_kt, hr2):
                # duplicate head rows into both partition halves so QK for two
                # consecutive k-tiles can run row-packed (tile_position 0/64)
                # concurrently in the PE array. One broadcast DMA per tile.
                tq = qthp.tile([P, nq], F32R, tag="qth", name="qth")
                nc.gpsimd.dma_start(tq[0:hd, :], src_qt[hr2:hr2 + hd, :])
                nc.gpsimd.dma_start(tq[hd:P, :], src_qt[hr2:hr2 + hd, :])
                tk = kthp.tile([P, nkv], F32R, tag="kth", name="kth")
                nc.gpsimd.dma_start(tk[0:hd, :], src_kt[hr2:hr2 + hd, :])
                nc.gpsimd.dma_start(tk[hd:P, :], src_kt[hr2:hr2 + hd, :])
                return tq, tk

            # kp slots (within a head) at which to emit the next pair's
            # QT / KT chunks, spread to avoid PSUM-slot bursts
            if npair >= 6:
                eslots = (1, 2, 3, 4)
            else:
                eslots = (0, 1, 2, 3)

            # ---------------- attention, projections interleaved ----------------
            cur_qt = cur_kt = None
            nxt_qt = nxt_kt = None
            qth_c = kth_c = None      # duplicated tiles for the current head
            qth_n = kth_n = None      # ... prepared one head ahead
            dup_even = min(5, npair - 2)   # slot preparing the odd head (from cur)
            dup_odd = min(5, npair - 1)    # slot preparing the next even head
            for hh in range(h):
                ht, hr = hh // hpt, (hh % hpt) * hd
                if hh == 0:
                    cur_qt = emit_qt(0)
                    cur_kt = ktp.tile([P, nkv], F32R, tag="kt", name="kt")
                    for kc2 in range(nkv // nq):
                        emit_kt_chunk(cur_kt, 0, kc2)
                    qth_c = kth_c = None  # head 0 runs unpacked (ramp)
                else:
                    if hh % hpt == 0:
                        cur_qt, cur_kt = nxt_qt, nxt_kt
                    qth_c, kth_c = qth_n, kth_n
                acc = ps_acc.tile([P, nq], F32, tag="acc", name="acc")
                for kp in range(npair):
                    if hh == 0:
                        if kp == 0:
                            emit_v_pair(0)
                        if kp + 1 < npair:
                            emit_v_pair(kp + 1)
                    if hh % hpt == 1 and (ht + 1) * hpt < h:
                        if kp == eslots[0]:
                            nxt_qt = emit_qt(ht + 1)
                            nxt_kt = ktp.tile([P, nkv], F32R, tag="kt", name="kt")
                        elif kp in eslots[1:]:
                            ei = eslots.index(kp) - 1
                            nchunk = nkv // nq
                            for kc2 in range(ei * nchunk // 3, (ei + 1) * nchunk // 3):
                                emit_kt_chunk(nxt_kt, ht + 1, kc2)
                    if hh + 1 < h:
                        if hh % hpt == 0 and kp == dup_even:
                            qth_n, kth_n = emit_dup(cur_qt, cur_kt, hd)
                        elif hh % hpt == 1 and kp == dup_odd:
                            qth_n, kth_n = emit_dup(nxt_qt, nxt_kt, 0)
                    # one 1 MB DMA covers two k-tiles of bias
                    bt = bias_pool.tile([P, 2 * nq], F16, tag="bias", name="bt")
                    nc.sync.dma_start(
                        bt[:].rearrange("p (t q) -> p t q", t=2),
                        biasT[hh, kp * 2 * P:(kp + 1) * 2 * P, :].rearrange(
                            "(t p) q -> p t q", t=2
                        ),
                    )
                    # exp runs shifted by -8 (cancels in the softmax ratio) so
                    # the fp16 ex tiles cannot overflow (scores reach ~12;
                    # exp(12) > fp16 max, exp(12-8) = 55).
                    # QK for the two k-tiles of the pair runs row-packed:
                    # ktl 0 on PE rows 0:64, ktl 1 on rows 64:128 (the head's
                    # data is duplicated in both halves of qth/kth), so the
                    # matmuls execute concurrently in the PE array.
                    pe_adds = [(kp * 2 + ktl) % pe_add_mod == 0 for ktl in range(2)]
                    if kth_c is not None:
                        scp = [
                            ps.tile([P, nq], F32, tag="ps", name="sc"),
                            ps.tile([P, nq], F32, tag="ps", name="sc2"),
                        ]
                        for qc in range(nQc):
                            for ktl in range(2):
                                kti = kp * 2 + ktl
                                rb = ktl * hd
                                nc.tensor.matmul(
                                    scp[ktl][:, qc * QF:(qc + 1) * QF],
                                    kth_c[rb:rb + hd, kti * P:(kti + 1) * P],
                                    qth_c[rb:rb + hd, qc * QF:(qc + 1) * QF],
                                    start=True,
                                    stop=not pe_adds[ktl],
                                )
                    else:
                        scp = []
                        for ktl in range(2):
                            kti = kp * 2 + ktl
                            sc0 = ps.tile([P, nq], F32, tag="ps", name="sc")
                            scp.append(sc0)
                            for qc in range(nQc):
                                nc.tensor.matmul(
                                    sc0[:, qc * QF:(qc + 1) * QF],
                                    cur_kt[hr:hr + hd, kti * P:(kti + 1) * P],
                                    cur_qt[hr:hr + hd, qc * QF:(qc + 1) * QF],
                                    start=True,
                                    stop=not pe_adds[ktl],
                                )
                    for ktl in range(2):
                        kti = kp * 2 + ktl
                        pe_add = pe_adds[ktl]
                        sc = scp[ktl]
                        if pe_add:
                            # bias += via identity matmul accumulate on PE
                            for qc in range(nQc):
                                nc.tensor.matmul(
                                    sc[:, qc * QF:(qc + 1) * QF],
                                    ident[:],
                                    bt[:, ktl * nq + qc * QF:
                                       ktl * nq + (qc + 1) * QF],
                                    start=False,
                                    stop=True,
                                )
                            ex_src = sc
                        else:
                            # add releases the PSUM slot early: sum goes to SBUF
                            sa = sadd_pool.tile([P, nq], F32, tag="sadd", name="sa")
                            nc.vector.tensor_add(
                                sa[:], sc[:], bt[:, ktl * nq:(ktl + 1) * nq]
                            )
                            ex_src = sa
                        ex = exp_pool.tile([P, nq], F16, tag="exp", name="ex")
                        nc.scalar.activation(
                            out=ex[:], in_=ex_src[:], func=EXP, bias=shift[:]
                        )
                        for qc in range(nQc):
                            nc.tensor.matmul(
                                acc[0:hw, qc * QF:(qc + 1) * QF],
                                vx[kti][:, hh * hw:(hh + 1) * hw],
                                ex[:, qc * QF:(qc + 1) * QF],
                                start=(kti == 0),
                                stop=(kti == nKt - 1),
                            )
                rec = small.tile([1, nq], F32, tag="rec", name="rec")
                nc.vector.reciprocal(rec[:], acc[hd:hw, :])
                bc = small.tile([hd, nq], F32, tag="bc", name="bc")
                nc.gpsimd.partition_broadcast(bc[:], rec[:])
                nc.vector.tensor_mul(
                    attnT[ht][hr:hr + hd, :], acc[0:hd, :], bc[:]
                )

            # ---------------- output projection ----------------
            # out[q, dm] = lhsT(attnT[dout, q]).T @ woT[dout, dm]
            for qt_i in range(nq // P):
                po = ps.tile([P, nq], F32, tag="ps", name="po")
                for dt in range(nDt):
                    nc.tensor.matmul(
                        po[:, 0:d],
                        attnT[dt][:, qt_i * P:(qt_i + 1) * P],
                        wo[dt][:],
                        start=(dt == 0),
                        stop=(dt == nDt - 1),
                    )
                ot = outp.tile([P, d], F32, tag="ot", name="ot")
                nc.vector.tensor_copy(ot[:], po[:, 0:d])
                nc.gpsimd.dma_start(out[qt_i * P:(qt_i + 1) * P, :], ot[:])

    nc.finalize()
    return nc


_NC_CACHE = {}


def _get_nc():
    if "nc" not in _NC_CACHE:
        _NC_CACHE["nc"] = build()
    return _NC_CACHE["nc"]


def _prep_inputs(x, enc, pos_bias, Wq, Wk, Wv, Wo):
    x = np.asarray(x, dtype=np.float32)
    enc = np.asarray(enc, dtype=np.float32)
    pos_bias = np.asarray(pos_bias, dtype=np.float32)
    shared = {
        "biasT": np.ascontiguousarray(pos_bias[0].transpose(0, 2, 1)).astype(np.float16),
        "wqT": np.ascontiguousarray(np.asarray(Wq, dtype=np.float32).T),
        "wkT": np.ascontiguousarray(np.asarray(Wk, dtype=np.float32).T),
        "wvT": np.ascontiguousarray(np.asarray(Wv, dtype=np.float32).T),
        "woT": np.ascontiguousarray(np.asarray(Wo, dtype=np.float32).T),
    }
    in_maps = []
    for b in range(B):
        m = dict(shared)
        m["xT"] = np.ascontiguousarray(x[b].T)
        m["encT"] = np.ascontiguousarray(enc[b].T)
        in_maps.append(m)
    return in_maps


def kernel_with_stats(x, enc, pos_bias, Wq, Wk, Wv, Wo, **run_kwargs):
    in_maps = _prep_inputs(x, enc, pos_bias, Wq, Wk, Wv, Wo)
    nc = _get_nc()
    res = run_bass_kernel_spmd(nc, in_maps, core_ids=list(range(B)), **run_kwargs)
    outs = np.stack([res.results[b]["out"] for b in range(B)], axis=0)
    return outs.astype(np.float32), res


def kernel(x, enc, pos_bias, Wq, Wk, Wv, Wo):
    outs, _ = kernel_with_stats(x, enc, pos_bias, Wq, Wk, Wv, Wo)
    return outs


# revision 45
# speedup vs baseline: 1.0163x; 1.0163x over previous
"""Encoder-decoder cross-attention (T5-style additive position bias) on 8 TRN2
NeuronCores.

Reference computation (per batch element b):
    Q = x @ Wq.T,  K = enc @ Wk.T,  V = enc @ Wv.T          (split into 8 heads of 64)
    scores = Q_h @ K_h.T + pos_bias[h]                       [NQ, NKV]
    out_h = softmax(scores) @ V_h
    out = concat_h(out_h) @ Wo.T

Sharding: pure data parallel over the batch (B=8 -> one batch element per core).
Each core streams the full 64 MB pos_bias (the dominant DMA term); compute is
sized under the DMA roofline, so the kernel targets the memory roofline.

Device-side layout: everything is computed transposed-by-head so the softmax'd
scores come out with the NKV (contraction) axis on partitions, feeding the
attn @ V matmul directly with zero on-chip transposes:
    QT[d, q] = Wq @ x.T        (host passes xT, WqT)
    KT[d, k] = Wk @ enc.T
    V[k, d]  = enc @ Wv.T      (+ a ones column per head -> softmax denominator)
    scT_h[k, q] = QK matmul accumulated in PSUM, bias added either by an
                  identity-matmul PSUM-accumulate on PE (1/4 of tiles) or a
                  DVE tensor_add into SBUF (3/4) to balance engine load.
                  The per-head Q/K rows are duplicated into both partition
                  halves (SWDGE copies, one head ahead) so the QK matmuls for
                  the two k-tiles of each pair run row-packed (tile_position
                  (0,0)/(64,0)) CONCURRENTLY in the PE array - the K=64
                  contraction would otherwise idle half the array
    accT_h[d(+1), q] = sum_k V'_h[k, d] * exp(scT_h[k, q])   (row 64 = denom)
    out[q, :] = (accT / denom) contracted with WoT

The QT/KT/V projections are EMITTED INTERLEAVED into the head loop (V inside
head 0 with one-pair prefetch, QT/KT for pair p spread across head 2p-1) —
Tile schedules engines in program-priority order, so this keeps the PE stream
and the PSUM slot queue flowing with the bias-stream pipeline instead of
front-running all projections.

QK and the projections run as float32r (~1.6e-4 max rel err measured on HW,
4x faster than float32); the position bias streams as fp16 (2^-11 quantization
of O(1) values, halving the dominant DMA term); exp outputs, V, attnT and Wo
are fp16 (softmax weights and O(0.5) values). The exp is computed with a
constant -8 shift folded into the ACT affine (cancels in the softmax ratio)
so fp16 exp cannot overflow; otherwise no max-subtraction pass is needed
(scores are O(10), far from fp32 exp range). Measured end-to-end vs the fp32
reference: ~6e-4 relative error.
"""

import os
import sys

for _p in ("/opt/trn_rl_repo", "/root/.axon_site/_ro/trn_rl_repo"):
    if _p not in sys.path and os.path.isdir(_p):
        sys.path.append(_p)

import numpy as np

import concourse.bass as bass
import concourse.tile as tile
from concourse import bacc, mybir
from concourse.bass_utils import run_bass_kernel_spmd
from concourse.masks import make_identity

B, NQ, NKV, D, H, HD = 8, 1024, 2048, 512, 8, 64
P = 128
F32 = mybir.dt.float32
F32R = mybir.dt.float32r
F16 = mybir.dt.float16
EXP = mybir.ActivationFunctionType.Exp


def build(nq=NQ, nkv=NKV, d=D, h=H, hd=HD, pe_add_mod=3):
    nDt = d // P          # dout partition tiles (4)
    nKt = nkv // P        # k partition tiles per head (16)
    QF = min(512, nq)     # matmul moving free-dim chunk
    nQc = nq // QF        # q chunks (2)
    KF = min(512, nkv)
    hpt = P // hd         # heads per 128-row tile (2)
    hw = hd + 1           # head width incl. ones column
    npair = nKt // 2

    nc = bacc.Bacc("TRN2")
    xT = nc.dram_tensor("xT", [d, nq], F32R, kind="ExternalInput")
    encT = nc.dram_tensor("encT", [d, nkv], F32R, kind="ExternalInput")
    biasT = nc.dram_tensor("biasT", [h, nkv, nq], F16, kind="ExternalInput")
    wqT = nc.dram_tensor("wqT", [d, d], F32R, kind="ExternalInput")
    wkT = nc.dram_tensor("wkT", [d, d], F32R, kind="ExternalInput")
    wvT = nc.dram_tensor("wvT", [d, d], F32R, kind="ExternalInput")
    woT = nc.dram_tensor("woT", [d, d], F32R, kind="ExternalInput")
    out = nc.dram_tensor("out", [nq, d], F32, kind="ExternalOutput")

    with tile.TileContext(nc) as tc:
        with (
            tc.tile_pool(name="persist", bufs=1) as persist,
            tc.tile_pool(name="qtp", bufs=2) as qtp,
            tc.tile_pool(name="qthp", bufs=2) as qthp,
            tc.tile_pool(name="kthp", bufs=2) as kthp,
            tc.tile_pool(name="ktp", bufs=2) as ktp,
            tc.tile_pool(name="bias", bufs=3) as bias_pool,
            tc.tile_pool(name="expp", bufs=4) as exp_pool,
            tc.tile_pool(name="sadd", bufs=4) as sadd_pool,
            tc.tile_pool(name="small", bufs=2) as small,
            tc.tile_pool(name="outp", bufs=3) as outp,
            tc.tile_pool(name="ps", bufs=3, space="PSUM") as ps,
            tc.tile_pool(name="ps_acc", bufs=1, space="PSUM") as ps_acc,
            tc.tile_pool(name="ph1", bufs=1) as ph1,
        ):
            vx = [persist.tile([P, h * hw], F16, name=f"vx{j}") for j in range(nKt)]
            wo = [persist.tile([P, d], F16, name=f"wo{j}") for j in range(nDt)]
            attnT = [persist.tile([P, nq], F16, name=f"attnT{j}") for j in range(nDt)]
            ident_f32 = persist.tile([P, P], F32, name="ident_f32")
            ident = persist.tile([P, P], F16, name="ident")
            make_identity(nc, ident_f32[:])
            nc.vector.tensor_copy(ident[:], ident_f32[:])

            xt = [ph1.tile([P, nq], F32R, name=f"xt{j}") for j in range(nDt)]
            et = [ph1.tile([P, nkv], F32R, name=f"et{j}") for j in range(nDt)]
            wq = [ph1.tile([P, d], F32R, name=f"wq{j}") for j in range(nDt)]
            wk = [ph1.tile([P, d], F32R, name=f"wk{j}") for j in range(nDt)]
            wv = [ph1.tile([P, d], F32R, name=f"wv{j}") for j in range(nDt)]
            ones_src = ph1.tile([P, h], F32, name="ones_src")
            nc.vector.memset(ones_src[:], 1.0)
            shift = persist.tile([P, 1], F32, name="shift")
            nc.vector.memset(shift[:], -8.0)

            # sync ring: small critical weights first, then the bias stream
            # owns it. scalar ring: xt+wv (needed ~10 us in). gpsimd SWDGE:
            # enc (overlaps the weight loads), wo (phase 3), output stores.
            for j in range(nDt):
                nc.sync.dma_start(wk[j][:], wkT[j * P:(j + 1) * P, :])
            for j in range(nDt):
                nc.sync.dma_start(wq[j][:], wqT[j * P:(j + 1) * P, :])
            for j in range(nDt):
                nc.gpsimd.dma_start(et[j][:], encT[j * P:(j + 1) * P, :])
            for j in range(nDt):
                nc.sync.dma_start(xt[j][:], xT[j * P:(j + 1) * P, :])
            for j in range(nDt):
                nc.scalar.dma_start(wv[j][:], wvT[j * P:(j + 1) * P, :])
            for j in range(nDt):
                nc.gpsimd.dma_start(wo[j][:], woT[j * P:(j + 1) * P, :])

            def emit_qt(dt):
                # QT[dout, q] = Wq @ xT : lhsT = wqT[din, dout], rhs = xT[din, q]
                t = qtp.tile([P, nq], F32R, tag="qt", name="qt")
                pt = ps.tile([P, nq], F32, tag="ps", name="pt")
                for k in range(nDt):
                    for qc in range(nQc):
                        nc.tensor.matmul(
                            pt[:, qc * QF:(qc + 1) * QF],
                            wq[k][:, dt * P:(dt + 1) * P],
                            xt[k][:, qc * QF:(qc + 1) * QF],
                            start=(k == 0),
                            stop=(k == nDt - 1),
                        )
                nc.vector.tensor_copy(t[:], pt[:])
                return t

            def emit_kt_chunk(t, dt, kc2):
                # KT[dout, k] = Wk @ encT, one nq-wide chunk of k
                pt = ps.tile([P, nq], F32, tag="ps", name="pt")
                for k in range(nDt):
                    for sub in range(nq // KF):
                        off = kc2 * nq + sub * KF
                        nc.tensor.matmul(
                            pt[:, sub * KF:(sub + 1) * KF],
                            wk[k][:, dt * P:(dt + 1) * P],
                            et[k][:, off:off + KF],
                            start=(k == 0),
                            stop=(k == nDt - 1),
                        )
                nc.vector.tensor_copy(t[:, kc2 * nq:(kc2 + 1) * nq], pt[:])

            def emit_v_pair(pi):
                # V[n, dout] = enc @ Wv.T for n-tiles 2*pi, 2*pi+1 (one psum tile)
                pt = ps.tile([P, nq], F32, tag="ps", name="pt")
                for half in range(2):
                    nt = 2 * pi + half
                    for k in range(nDt):
                        nc.tensor.matmul(
                            pt[:, half * d:half * d + d],
                            et[k][:, nt * P:(nt + 1) * P],
                            wv[k][:],
                            start=(k == 0),
                            stop=(k == nDt - 1),
                        )
                for half in range(2):
                    nt = 2 * pi + half
                    vg = vx[nt][:].rearrange("p (h e) -> p h e", e=hw)
                    nc.vector.tensor_copy(
                        vg[:, :, hd:hw], ones_src[:].unsqueeze(-1)
                    )
                    nc.vector.tensor_copy(
                        vg[:, :, 0:hd],
                        pt[:, half * d:half * d + d].rearrange(
                            "p (h e) -> p h e", e=hd
                        ),
                    )

            def emit_dup(src_qt, src_kt, hr2):
                # duplicate head rows into both partition halves so QK for two
                # consecutive k-tiles can run row-packed (tile_position 0/64)
                # concurrently in the PE array. One broadcast DMA per tile.
                tq = qthp.tile([P, nq], F32R, tag="qth", name="qth")
                nc.gpsimd.dma_start(tq[0:hd, :], src_qt[hr2:hr2 + hd, :])
                nc.gpsimd.dma_start(tq[hd:P, :], src_qt[hr2:hr2 + hd, :])
                tk = kthp.tile([P, nkv], F32R, tag="kth", name="kth")
                nc.gpsimd.dma_start(tk[0:hd, :], src_kt[hr2:hr2 + hd, :])
                nc.gpsimd.dma_start(tk[hd:P, :], src_kt[hr2:hr2 + hd, :])
                return tq, tk

            # kp slots (within a head) at which to emit the next pair's
            # QT / KT chunks, spread to avoid PSUM-slot bursts
            if npair >= 6:
                eslots = (1, 2, 3, 4)
            else:
                eslots = (0, 1, 2, 3)

            # ---------------- attention, projections interleaved ----------------
            cur_qt = cur_kt = None
            nxt_qt = nxt_kt = None
            qth_c = kth_c = None      # duplicated tiles for the current head
            qth_n = kth_n = None      # ... prepared one head ahead
            dup_even = min(5, npair - 2)   # slot preparing the odd head (from cur)
            dup_odd = min(5, npair - 1)    # slot preparing the next even head
            for hh in range(h):
                ht, hr = hh // hpt, (hh % hpt) * hd
                if hh == 0:
                    cur_qt = emit_qt(0)
                    cur_kt = ktp.tile([P, nkv], F32R, tag="kt", name="kt")
                    for kc2 in range(nkv // nq):
                        emit_kt_chunk(cur_kt, 0, kc2)
                    qth_c = kth_c = None  # head 0 runs unpacked (ramp)
                else:
                    if hh % hpt == 0:
                        cur_qt, cur_kt = nxt_qt, nxt_kt
                    qth_c, kth_c = qth_n, kth_n
                acc = ps_acc.tile([P, nq], F32, tag="acc", name="acc")
                for kp in range(npair):
                    if hh == 0:
                        if kp == 0:
                            emit_v_pair(0)
                        if kp + 1 < npair:
                            emit_v_pair(kp + 1)
                    if hh % hpt == 1 and (ht + 1) * hpt < h:
                        if kp == eslots[0]:
                            nxt_qt = emit_qt(ht + 1)
                            nxt_kt = ktp.tile([P, nkv], F32R, tag="kt", name="kt")
                        elif kp in eslots[1:]:
                            ei = eslots.index(kp) - 1
                            nchunk = nkv // nq
                            for kc2 in range(ei * nchunk // 3, (ei + 1) * nchunk // 3):
                                emit_kt_chunk(nxt_kt, ht + 1, kc2)
                    if hh + 1 < h:
                        if hh % hpt == 0 and kp == dup_even:
                            qth_n, kth_n = emit_dup(cur_qt, cur_kt, hd)
                        elif hh % hpt == 1 and kp == dup_odd:
                            qth_n, kth_n = emit_dup(nxt_qt, nxt_kt, 0)
                    # one 1 MB DMA covers two k-tiles of bias
                    bt = bias_pool.tile([P, 2 * nq], F16, tag="bias", name="bt")
                    nc.sync.dma_start(
                        bt[:].rearrange("p (t q) -> p t q", t=2),
                        biasT[hh, kp * 2 * P:(kp + 1) * 2 * P, :].rearrange(
                            "(t p) q -> p t q", t=2
                        ),
                    )
                    # exp runs shifted by -8 (cancels in the softmax ratio) so
                    # the fp16 ex tiles cannot overflow (scores reach ~12;
                    # exp(12) > fp16 max, exp(12-8) = 55).
                    # QK for the two k-tiles of the pair runs row-packed:
                    # ktl 0 on PE rows 0:64, ktl 1 on rows 64:128 (the head's
                    # data is duplicated in both halves of qth/kth), so the
                    # matmuls execute concurrently in the PE array.
                    pe_adds = [(kp * 2 + ktl) % pe_add_mod == 0 for ktl in range(2)]
                    if kth_c is not None:
                        scp = [
                            ps.tile([P, nq], F32, tag="ps", name="sc"),
                            ps.tile([P, nq], F32, tag="ps", name="sc2"),
                        ]
                        for qc in range(nQc):
                            for ktl in range(2):
                                kti = kp * 2 + ktl
                                rb = ktl * hd
                                nc.tensor.matmul(
                                    scp[ktl][:, qc * QF:(qc + 1) * QF],
                                    kth_c[rb:rb + hd, kti * P:(kti + 1) * P],
                                    qth_c[rb:rb + hd, qc * QF:(qc + 1) * QF],
                                    start=True,
                                    stop=not pe_adds[ktl],
                                )
                    else:
                        scp = []
                        for ktl in range(2):
                            kti = kp * 2 + ktl
                            sc0 = ps.tile([P, nq], F32, tag="ps", name="sc")
                            scp.append(sc0)
                            for qc in range(nQc):
                                nc.tensor.matmul(
                                    sc0[:, qc * QF:(qc + 1) * QF],
                                    cur_kt[hr:hr + hd, kti * P:(kti + 1) * P],
                                    cur_qt[hr:hr + hd, qc * QF:(qc + 1) * QF],
                                    start=True,
                                    stop=not pe_adds[ktl],
                                )
                    for ktl in range(2):
                        kti = kp * 2 + ktl
                        pe_add = pe_adds[ktl]
                        sc = scp[ktl]
                        if pe_add:
                            # bias += via identity matmul accumulate on PE
                            for qc in range(nQc):
                                nc.tensor.matmul(
                                    sc[:, qc * QF:(qc + 1) * QF],
                                    ident[:],
                                    bt[:, ktl * nq + qc * QF:
                                       ktl * nq + (qc + 1) * QF],
                                    start=False,
                                    stop=True,
                                )
                            ex_src = sc
                        else:
                            # add releases the PSUM slot early: sum goes to SBUF
                            sa = sadd_pool.tile([P, nq], F32, tag="sadd", name="sa")
                            nc.vector.tensor_add(
                                sa[:], sc[:], bt[:, ktl * nq:(ktl + 1) * nq]
                            )
                            ex_src = sa
                        ex = exp_pool.tile([P, nq], F16, tag="exp", name="ex")
                        nc.scalar.activation(
                            out=ex[:], in_=ex_src[:], func=EXP, bias=shift[:]
                        )
                        for qc in range(nQc):
                            nc.tensor.matmul(
                                acc[0:hw, qc * QF:(qc + 1) * QF],
                                vx[kti][:, hh * hw:(hh + 1) * hw],
                                ex[:, qc * QF:(qc + 1) * QF],
                                start=(kti == 0),
                                stop=(kti == nKt - 1),
                            )
                rec = small.tile([1, nq], F32, tag="rec", name="rec")
                nc.vector.reciprocal(rec[:], acc[hd:hw, :])
                bc = small.tile([hd, nq], F32, tag="bc", name="bc")
                nc.gpsimd.partition_broadcast(bc[:], rec[:])
                nc.vector.tensor_mul(
                    attnT[ht][hr:hr + hd, :], acc[0:hd, :], bc[:]
                )

            # ---------------- output projection ----------------
            # out[q, dm] = lhsT(attnT[dout, q]).T @ woT[dout, dm]
            for qt_i in range(nq // P):
                po = ps.tile([P, nq], F32, tag="ps", name="po")
                for dt in range(nDt):
                    nc.tensor.matmul(
                        po[:, 0:d],
                        attnT[dt][:, qt_i * P:(qt_i + 1) * P],
                        wo[dt][:],
                        start=(dt == 0),
                        stop=(dt == nDt - 1),
                    )
                ot = outp.tile([P, d], F32, tag="ot", name="ot")
                nc.vector.tensor_copy(ot[:], po[:, 0:d])
                nc.gpsimd.dma_start(out[qt_i * P:(qt_i + 1) * P, :], ot[:])

    nc.finalize()
    return nc


_NC_CACHE = {}


def _get_nc():
    if "nc" not in _NC_CACHE:
        _NC_CACHE["nc"] = build()
    return _NC_CACHE["nc"]


def _prep_inputs(x, enc, pos_bias, Wq, Wk, Wv, Wo):
    x = np.asarray(x, dtype=np.float32)
    enc = np.asarray(enc, dtype=np.float32)
    pos_bias = np.asarray(pos_bias, dtype=np.float32)
    shared = {
        "biasT": np.ascontiguousarray(pos_bias[0].transpose(0, 2, 1)).astype(np.float16),
        "wqT": np.ascontiguousarray(np.asarray(Wq, dtype=np.float32).T),
        "wkT": np.ascontiguousarray(np.asarray(Wk, dtype=np.float32).T),
        "wvT": np.ascontiguousarray(np.asarray(Wv, dtype=np.float32).T),
        "woT": np.ascontiguousarray(np.asarray(Wo, dtype=np.float32).T),
    }
    in_maps = []
    for b in range(B):
        m = dict(shared)
        m["xT"] = np.ascontiguousarray(x[b].T)
        m["encT"] = np.ascontiguousarray(enc[b].T)
        in_maps.append(m)
    return in_maps


def kernel_with_stats(x, enc, pos_bias, Wq, Wk, Wv, Wo, **run_kwargs):
    in_maps = _prep_inputs(x, enc, pos_bias, Wq, Wk, Wv, Wo)
    nc = _get_nc()
    res = run_bass_kernel_spmd(nc, in_maps, core_ids=list(range(B)), **run_kwargs)
    outs = np.stack([res.results[b]["out"] for b in range(B)], axis=0)
    return outs.astype(np.float32), res


def kernel(x, enc, pos_bias, Wq, Wk, Wv, Wo):
    outs, _ = kernel_with_stats(x, enc, pos_bias, Wq, Wk, Wv, Wo)
    return outs


# revision 49
# speedup vs baseline: 1.0305x; 1.0139x over previous
"""Encoder-decoder cross-attention (T5-style additive position bias) on 8 TRN2
NeuronCores.

Reference computation (per batch element b):
    Q = x @ Wq.T,  K = enc @ Wk.T,  V = enc @ Wv.T          (split into 8 heads of 64)
    scores = Q_h @ K_h.T + pos_bias[h]                       [NQ, NKV]
    out_h = softmax(scores) @ V_h
    out = concat_h(out_h) @ Wo.T

Sharding: pure data parallel over the batch (B=8 -> one batch element per core).
Each core streams the full 64 MB pos_bias (the dominant DMA term); compute is
sized under the DMA roofline, so the kernel targets the memory roofline.

Device-side layout: everything is computed transposed-by-head so the softmax'd
scores come out with the NKV (contraction) axis on partitions, feeding the
attn @ V matmul directly with zero on-chip transposes:
    QT[d, q] = Wq @ x.T        (host passes xT, WqT)
    KT[d, k] = Wk @ enc.T
    V[k, d]  = enc @ Wv.T      (+ a ones column per head -> softmax denominator)
    scT_h[k, q] = QK matmul accumulated in PSUM, bias added either by an
                  identity-matmul PSUM-accumulate on PE (1/4 of tiles) or a
                  DVE tensor_add into SBUF (3/4) to balance engine load.
                  The per-head Q/K rows are duplicated into both partition
                  halves (SWDGE copies, one head ahead) so the QK matmuls for
                  the two k-tiles of each pair run row-packed (tile_position
                  (0,0)/(64,0)) CONCURRENTLY in the PE array - the K=64
                  contraction would otherwise idle half the array
    accT_h[d(+1), q] = sum_k V'_h[k, d] * exp(scT_h[k, q])   (row 64 = denom)
    out[q, :] = (accT / denom) contracted with WoT

The QT/KT/V projections are EMITTED INTERLEAVED into the head loop (V inside
head 0 with one-pair prefetch, QT/KT for pair p spread across head 2p-1) —
Tile schedules engines in program-priority order, so this keeps the PE stream
and the PSUM slot queue flowing with the bias-stream pipeline instead of
front-running all projections.

QK and the projections run as float32r (~1.6e-4 max rel err measured on HW,
4x faster than float32); the position bias streams as fp16 (2^-11 quantization
of O(1) values, halving the dominant DMA term); exp outputs, V, attnT and Wo
are fp16 (softmax weights and O(0.5) values). The exp is computed with a
constant -8 shift folded into the ACT affine (cancels in the softmax ratio)
so fp16 exp cannot overflow; otherwise no max-subtraction pass is needed
(scores are O(10), far from fp32 exp range). Measured end-to-end vs the fp32
reference: ~6e-4 relative error.
"""

import os
import sys

for _p in ("/opt/trn_rl_repo", "/root/.axon_site/_ro/trn_rl_repo"):
    if _p not in sys.path and os.path.isdir(_p):
        sys.path.append(_p)

import numpy as np

import concourse.bass as bass
import concourse.tile as tile
from concourse import bacc, mybir
from concourse.bass_utils import run_bass_kernel_spmd
from concourse.masks import make_identity

B, NQ, NKV, D, H, HD = 8, 1024, 2048, 512, 8, 64
P = 128
F32 = mybir.dt.float32
F32R = mybir.dt.float32r
F16 = mybir.dt.float16
EXP = mybir.ActivationFunctionType.Exp


def build(nq=NQ, nkv=NKV, d=D, h=H, hd=HD, pe_add_mod=3):
    nDt = d // P          # dout partition tiles (4)
    nKt = nkv // P        # k partition tiles per head (16)
    QF = min(512, nq)     # matmul moving free-dim chunk
    nQc = nq // QF        # q chunks (2)
    KF = min(512, nkv)
    hpt = P // hd         # heads per 128-row tile (2)
    hw = hd + 1           # head width incl. ones column
    npair = nKt // 2

    nc = bacc.Bacc("TRN2")
    xT = nc.dram_tensor("xT", [d, nq], F32R, kind="ExternalInput")
    encT = nc.dram_tensor("encT", [d, nkv], F32R, kind="ExternalInput")
    biasT = nc.dram_tensor("biasT", [h, nkv, nq], F16, kind="ExternalInput")
    wqT = nc.dram_tensor("wqT", [d, d], F32R, kind="ExternalInput")
    wkT = nc.dram_tensor("wkT", [d, d], F32R, kind="ExternalInput")
    wvT = nc.dram_tensor("wvT", [d, d], F32R, kind="ExternalInput")
    woT = nc.dram_tensor("woT", [d, d], F32R, kind="ExternalInput")
    out = nc.dram_tensor("out", [nq, d], F32, kind="ExternalOutput")

    with tile.TileContext(nc) as tc:
        with (
            tc.tile_pool(name="persist", bufs=1) as persist,
            tc.tile_pool(name="qtp", bufs=2) as qtp,
            tc.tile_pool(name="qthp", bufs=2) as qthp,
            tc.tile_pool(name="kthp", bufs=2) as kthp,
            tc.tile_pool(name="ktp", bufs=2) as ktp,
            tc.tile_pool(name="bias", bufs=3) as bias_pool,
            tc.tile_pool(name="expp", bufs=4) as exp_pool,
            tc.tile_pool(name="sadd", bufs=4) as sadd_pool,
            tc.tile_pool(name="small", bufs=2) as small,
            tc.tile_pool(name="outp", bufs=3) as outp,
            tc.tile_pool(name="ps", bufs=3, space="PSUM") as ps,
            tc.tile_pool(name="ps_acc", bufs=1, space="PSUM") as ps_acc,
            tc.tile_pool(name="ph1", bufs=1) as ph1,
        ):
            vx = [persist.tile([P, h * hw], F16, name=f"vx{j}") for j in range(nKt)]
            wo = [persist.tile([P, d], F16, name=f"wo{j}") for j in range(nDt)]
            attnT = [persist.tile([P, nq], F16, name=f"attnT{j}") for j in range(nDt)]
            ident_f32 = persist.tile([P, P], F32, name="ident_f32")
            ident = persist.tile([P, P], F16, name="ident")
            make_identity(nc, ident_f32[:])
            nc.vector.tensor_copy(ident[:], ident_f32[:])

            xt = [ph1.tile([P, nq], F32R, name=f"xt{j}") for j in range(nDt)]
            et = [ph1.tile([P, nkv], F32R, name=f"et{j}") for j in range(nDt)]
            wq = [ph1.tile([P, d], F32R, name=f"wq{j}") for j in range(nDt)]
            wk = [ph1.tile([P, d], F32R, name=f"wk{j}") for j in range(nDt)]
            wv = [ph1.tile([P, d], F32R, name=f"wv{j}") for j in range(nDt)]
            ones_src = ph1.tile([P, h], F32, name="ones_src")
            nc.vector.memset(ones_src[:], 1.0)
            shift = persist.tile([P, 1], F32, name="shift")
            nc.vector.memset(shift[:], -8.0)

            # sync ring: small critical weights first, then the bias stream
            # owns it. scalar ring: xt+wv (needed ~10 us in). gpsimd SWDGE:
            # enc (overlaps the weight loads), wo (phase 3), output stores.
            for j in range(nDt):
                nc.sync.dma_start(wk[j][:], wkT[j * P:(j + 1) * P, :])
            for j in range(nDt):
                nc.sync.dma_start(wq[j][:], wqT[j * P:(j + 1) * P, :])
            for j in range(nDt):
                nc.gpsimd.dma_start(et[j][:], encT[j * P:(j + 1) * P, :])
            for j in range(nDt):
                nc.sync.dma_start(xt[j][:], xT[j * P:(j + 1) * P, :])
            for j in range(nDt):
                nc.scalar.dma_start(wv[j][:], wvT[j * P:(j + 1) * P, :])
            for j in range(nDt):
                nc.gpsimd.dma_start(wo[j][:], woT[j * P:(j + 1) * P, :])

            def emit_qt(dt):
                # QT[dout, q] = Wq @ xT : lhsT = wqT[din, dout], rhs = xT[din, q]
                t = qtp.tile([P, nq], F32R, tag="qt", name="qt")
                pt = ps.tile([P, nq], F32, tag="ps", name="pt")
                for k in range(nDt):
                    for qc in range(nQc):
                        nc.tensor.matmul(
                            pt[:, qc * QF:(qc + 1) * QF],
                            wq[k][:, dt * P:(dt + 1) * P],
                            xt[k][:, qc * QF:(qc + 1) * QF],
                            start=(k == 0),
                            stop=(k == nDt - 1),
                        )
                nc.vector.tensor_copy(t[:], pt[:])
                return t

            def emit_kt_chunk(t, dt, kc2):
                # KT[dout, k] = Wk @ encT, one nq-wide chunk of k
                pt = ps.tile([P, nq], F32, tag="ps", name="pt")
                for k in range(nDt):
                    for sub in range(nq // KF):
                        off = kc2 * nq + sub * KF
                        nc.tensor.matmul(
                            pt[:, sub * KF:(sub + 1) * KF],
                            wk[k][:, dt * P:(dt + 1) * P],
                            et[k][:, off:off + KF],
                            start=(k == 0),
                            stop=(k == nDt - 1),
                        )
                nc.vector.tensor_copy(t[:, kc2 * nq:(kc2 + 1) * nq], pt[:])

            def emit_v_pair(pi):
                # V[n, dout] = enc @ Wv.T for n-tiles 2*pi, 2*pi+1 (one psum tile)
                pt = ps.tile([P, nq], F32, tag="ps", name="pt")
                for half in range(2):
                    nt = 2 * pi + half
                    for k in range(nDt):
                        nc.tensor.matmul(
                            pt[:, half * d:half * d + d],
                            et[k][:, nt * P:(nt + 1) * P],
                            wv[k][:],
                            start=(k == 0),
                            stop=(k == nDt - 1),
                        )
                for half in range(2):
                    nt = 2 * pi + half
                    vg = vx[nt][:].rearrange("p (h e) -> p h e", e=hw)
                    nc.vector.tensor_copy(
                        vg[:, :, hd:hw], ones_src[:].unsqueeze(-1)
                    )
                    nc.vector.tensor_copy(
                        vg[:, :, 0:hd],
                        pt[:, half * d:half * d + d].rearrange(
                            "p (h e) -> p h e", e=hd
                        ),
                    )

            def emit_dup(src_qt, src_kt, hr2):
                # duplicate head rows into both partition halves so QK for two
                # consecutive k-tiles can run row-packed (tile_position 0/64)
                # concurrently in the PE array. One broadcast DMA per tile.
                tq = qthp.tile([P, nq], F32R, tag="qth", name="qth")
                nc.gpsimd.dma_start(tq[0:hd, :], src_qt[hr2:hr2 + hd, :])
                nc.gpsimd.dma_start(tq[hd:P, :], src_qt[hr2:hr2 + hd, :])
                tk = kthp.tile([P, nkv], F32R, tag="kth", name="kth")
                nc.gpsimd.dma_start(tk[0:hd, :], src_kt[hr2:hr2 + hd, :])
                nc.gpsimd.dma_start(tk[hd:P, :], src_kt[hr2:hr2 + hd, :])
                return tq, tk

            # kp slots (within a head) at which to emit the next pair's
            # QT / KT chunks, spread to avoid PSUM-slot bursts
            if npair >= 6:
                eslots = (1, 2, 3, 4)
            else:
                eslots = (0, 1, 2, 3)

            # ---------------- attention, projections interleaved ----------------
            cur_qt = cur_kt = None
            nxt_qt = nxt_kt = None
            qth_c = kth_c = None      # duplicated tiles for the current head
            qth_n = kth_n = None      # ... prepared one head ahead
            dup_even = min(5, npair - 2)   # slot preparing the odd head (from cur)
            dup_odd = min(5, npair - 1)    # slot preparing the next even head
            for hh in range(h):
                ht, hr = hh // hpt, (hh % hpt) * hd
                if hh == 0:
                    cur_qt = emit_qt(0)
                    cur_kt = ktp.tile([P, nkv], F32R, tag="kt", name="kt")
                    for kc2 in range(nkv // nq):
                        emit_kt_chunk(cur_kt, 0, kc2)
                    qth_c = kth_c = None  # head 0 runs unpacked (ramp)
                else:
                    if hh % hpt == 0:
                        cur_qt, cur_kt = nxt_qt, nxt_kt
                    qth_c, kth_c = qth_n, kth_n
                acc = ps_acc.tile([P, nq], F32, tag="acc", name="acc")
                for kp in range(npair):
                    if hh == 0:
                        if kp == 0:
                            emit_v_pair(0)
                        if kp + 1 < npair:
                            emit_v_pair(kp + 1)
                    if hh == 1 or (npair < 6 and hh % hpt == 1):
                        # first pair's successor: all emissions in head 1
                        if (ht + 1) * hpt < h:
                            if kp == eslots[0]:
                                nxt_qt = emit_qt(ht + 1)
                                nxt_kt = ktp.tile([P, nkv], F32R, tag="kt", name="kt")
                            elif kp in eslots[1:]:
                                ei = eslots.index(kp) - 1
                                nchunk = nkv // nq
                                for kc2 in range(ei * nchunk // 3,
                                                 (ei + 1) * nchunk // 3):
                                    emit_kt_chunk(nxt_kt, ht + 1, kc2)
                    elif npair >= 6 and (ht + 1) * hpt < h:
                        # later pairs: spread emissions across BOTH heads of
                        # the pair to halve the per-head burst (QT + KT chunk 0
                        # in the even head, chunk 1 early in the odd head; the
                        # kp5 dup still follows every chunk write in program
                        # order)
                        if hh % hpt == 0 and hh >= 2:
                            if kp == 3:
                                nxt_qt = emit_qt(ht + 1)
                                nxt_kt = ktp.tile([P, nkv], F32R, tag="kt",
                                                  name="kt")
                            elif kp == 6:
                                emit_kt_chunk(nxt_kt, ht + 1, 0)
                        elif hh % hpt == 1 and hh >= 3:
                            if kp == 2:
                                for kc2 in range(1, nkv // nq):
                                    emit_kt_chunk(nxt_kt, ht + 1, kc2)
                    if hh + 1 < h:
                        if hh % hpt == 0 and kp == dup_even:
                            qth_n, kth_n = emit_dup(cur_qt, cur_kt, hd)
                        elif hh % hpt == 1 and kp == dup_odd:
                            qth_n, kth_n = emit_dup(nxt_qt, nxt_kt, 0)
                    # one 1 MB DMA covers two k-tiles of bias
                    bt = bias_pool.tile([P, 2 * nq], F16, tag="bias", name="bt")
                    nc.sync.dma_start(
                        bt[:].rearrange("p (t q) -> p t q", t=2),
                        biasT[hh, kp * 2 * P:(kp + 1) * 2 * P, :].rearrange(
                            "(t p) q -> p t q", t=2
                        ),
                    )
                    # exp runs shifted by -8 (cancels in the softmax ratio) so
                    # the fp16 ex tiles cannot overflow (scores reach ~12;
                    # exp(12) > fp16 max, exp(12-8) = 55).
                    # QK for the two k-tiles of the pair runs row-packed:
                    # ktl 0 on PE rows 0:64, ktl 1 on rows 64:128 (the head's
                    # data is duplicated in both halves of qth/kth), so the
                    # matmuls execute concurrently in the PE array.
                    pe_adds = [(kp * 2 + ktl) % pe_add_mod == 0 for ktl in range(2)]
                    if kth_c is not None:
                        scp = [
                            ps.tile([P, nq], F32, tag="ps", name="sc"),
                            ps.tile([P, nq], F32, tag="ps", name="sc2"),
                        ]
                        for qc in range(nQc):
                            for ktl in range(2):
                                kti = kp * 2 + ktl
                                rb = ktl * hd
                                nc.tensor.matmul(
                                    scp[ktl][:, qc * QF:(qc + 1) * QF],
                                    kth_c[rb:rb + hd, kti * P:(kti + 1) * P],
                                    qth_c[rb:rb + hd, qc * QF:(qc + 1) * QF],
                                    start=True,
                                    stop=not pe_adds[ktl],
                                )
                    else:
                        scp = []
                        for ktl in range(2):
                            kti = kp * 2 + ktl
                            sc0 = ps.tile([P, nq], F32, tag="ps", name="sc")
                            scp.append(sc0)
                            for qc in range(nQc):
                                nc.tensor.matmul(
                                    sc0[:, qc * QF:(qc + 1) * QF],
                                    cur_kt[hr:hr + hd, kti * P:(kti + 1) * P],
                                    cur_qt[hr:hr + hd, qc * QF:(qc + 1) * QF],
                                    start=True,
                                    stop=not pe_adds[ktl],
                                )
                    for ktl in range(2):
                        kti = kp * 2 + ktl
                        pe_add = pe_adds[ktl]
                        sc = scp[ktl]
                        if pe_add:
                            # bias += via identity matmul accumulate on PE
                            for qc in range(nQc):
                                nc.tensor.matmul(
                                    sc[:, qc * QF:(qc + 1) * QF],
                                    ident[:],
                                    bt[:, ktl * nq + qc * QF:
                                       ktl * nq + (qc + 1) * QF],
                                    start=False,
                                    stop=True,
                                )
                            ex_src = sc
                        else:
                            # add releases the PSUM slot early: sum goes to SBUF
                            sa = sadd_pool.tile([P, nq], F32, tag="sadd", name="sa")
                            nc.vector.tensor_add(
                                sa[:], sc[:], bt[:, ktl * nq:(ktl + 1) * nq]
                            )
                            ex_src = sa
                        ex = exp_pool.tile([P, nq], F16, tag="exp", name="ex")
                        nc.scalar.activation(
                            out=ex[:], in_=ex_src[:], func=EXP, bias=shift[:]
                        )
                        for qc in range(nQc):
                            nc.tensor.matmul(
                                acc[0:hw, qc * QF:(qc + 1) * QF],
                                vx[kti][:, hh * hw:(hh + 1) * hw],
                                ex[:, qc * QF:(qc + 1) * QF],
                                start=(kti == 0),
                                stop=(kti == nKt - 1),
                            )
                rec = small.tile([1, nq], F32, tag="rec", name="rec")
                nc.vector.reciprocal(rec[:], acc[hd:hw, :])
                bc = small.tile([hd, nq], F32, tag="bc", name="bc")
                nc.gpsimd.partition_broadcast(bc[:], rec[:])
                nc.vector.tensor_mul(
                    attnT[ht][hr:hr + hd, :], acc[0:hd, :], bc[:]
                )

            # ---------------- output projection ----------------
            # out[q, dm] = lhsT(attnT[dout, q]).T @ woT[dout, dm]
            for qt_i in range(nq // P):
                po = ps.tile([P, nq], F32, tag="ps", name="po")
                for dt in range(nDt):
                    nc.tensor.matmul(
                        po[:, 0:d],
                        attnT[dt][:, qt_i * P:(qt_i + 1) * P],
                        wo[dt][:],
                        start=(dt == 0),
                        stop=(dt == nDt - 1),
                    )
                ot = outp.tile([P, d], F32, tag="ot", name="ot")
                nc.vector.tensor_copy(ot[:], po[:, 0:d])
                nc.gpsimd.dma_start(out[qt_i * P:(qt_i + 1) * P, :], ot[:])

    nc.finalize()
    return nc


_NC_CACHE = {}


def _get_nc():
    if "nc" not in _NC_CACHE:
        _NC_CACHE["nc"] = build()
    return _NC_CACHE["nc"]


def _prep_inputs(x, enc, pos_bias, Wq, Wk, Wv, Wo):
    x = np.asarray(x, dtype=np.float32)
    enc = np.asarray(enc, dtype=np.float32)
    pos_bias = np.asarray(pos_bias, dtype=np.float32)
    shared = {
        "biasT": np.ascontiguousarray(pos_bias[0].transpose(0, 2, 1)).astype(np.float16),
        "wqT": np.ascontiguousarray(np.asarray(Wq, dtype=np.float32).T),
        "wkT": np.ascontiguousarray(np.asarray(Wk, dtype=np.float32).T),
        "wvT": np.ascontiguousarray(np.asarray(Wv, dtype=np.float32).T),
        "woT": np.ascontiguousarray(np.asarray(Wo, dtype=np.float32).T),
    }
    in_maps = []
    for b in range(B):
        m = dict(shared)
        m["xT"] = np.ascontiguousarray(x[b].T)
        m["encT"] = np.ascontiguousarray(enc[b].T)
        in_maps.append(m)
    return in_maps


def kernel_with_stats(x, enc, pos_bias, Wq, Wk, Wv, Wo, **run_kwargs):
    in_maps = _prep_inputs(x, enc, pos_bias, Wq, Wk, Wv, Wo)
    nc = _get_nc()
    res = run_bass_kernel_spmd(nc, in_maps, core_ids=list(range(B)), **run_kwargs)
    outs = np.stack([res.results[b]["out"] for b in range(B)], axis=0)
    return outs.astype(np.float32), res


def kernel(x, enc, pos_bias, Wq, Wk, Wv, Wo):
    outs, _ = kernel_with_stats(x, enc, pos_bias, Wq, Wk, Wv, Wo)
    return outs
